# revision 1
# baseline (speedup 1.0000x reference)
"""Causal self-attention on 8 Trainium2 NeuronCores.

Sharding: core c = 2*b + g handles batch b (of 4) and head-group g (8 of 16
heads). Each core computes the qkv projection for its head slice, causal
attention for its 8 heads, and the output projection against its 512-row
slice of w_proj. The two half-projections per batch are summed on the host
(plus b_proj).

Per-core layout:
  Q^T, K^T [128, 2048] feature-major per pair of heads (rotating pool tiles)
  V [2048, 768] token-major as [V_even | ones | V_odd] per head pair: the AV
    stationary [V_h | ones] (or [ones | V_h]) yields O^T rows plus the
    softmax denominator replicated over the other 64 partitions in one shot.
  S^T = K @ Q^T per (head, 512-wide q super-block): no transposes anywhere,
    masking is one additive -1e9 region per diagonal-block chunk before exp.
  Attention for pair p is interleaved with the Q/K projection of pair p+1 so
  the PE always has dependency-free matmuls while ACT computes exp.
All matmuls run in float32r (TF32-like, ~1.5e-4 rel err).
"""

import sys

sys.path.insert(0, "/opt/trn_rl_repo")

import numpy as np

import concourse.bass as bass
import concourse.mybir as mybir
import concourse.tile as tile
from concourse import bacc
from concourse.bass import AP

F32 = mybir.dt.float32
F32R = mybir.dt.float32r
AF = mybir.ActivationFunctionType

N_CORES = 8
T = 2048
C = 1024
D = 64
P = 128
NT = T // P     # 16 token chunks
NS = 4          # q super-blocks of 512
CK = C // P     # 8 contraction chunks
NEG = -1e9


def build_nc(repeat: int = 1, timing: bool = False):
    nc = bacc.Bacc("TRN2", target_bir_lowering=False, debug=False)

    xt = nc.dram_tensor("xt", [C, T], F32, kind="ExternalInput").ap()
    wq = nc.dram_tensor("wq", [C, 512], F32, kind="ExternalInput").ap()
    wk = nc.dram_tensor("wk", [C, 512], F32, kind="ExternalInput").ap()
    wv = nc.dram_tensor("wv", [C, 512], F32, kind="ExternalInput").ap()
    wp = nc.dram_tensor("wp", [512, C], F32, kind="ExternalInput").ap()
    bq = nc.dram_tensor("bq", [512, 1], F32, kind="ExternalInput").ap()
    bk = nc.dram_tensor("bk", [512, 1], F32, kind="ExternalInput").ap()
    bv = nc.dram_tensor("bv", [1, 512], F32, kind="ExternalInput").ap()
    # mask[:, r*512:(r+1)*512] is the additive mask for the diagonal-super
    # chunk with within-super row index r (cols 0..r*128 fully -1e9, block r
    # strictly-lower-triangle -1e9, rest 0)
    mask = nc.dram_tensor("mask", [P, 4 * 512], F32, kind="ExternalInput").ap()
    if timing:
        out = nc.dram_tensor("out", [T, C], F32).ap()
        probe = nc.dram_tensor("probe", [P, 4], F32, kind="ExternalOutput").ap()
    else:
        out = nc.dram_tensor("out", [T, C], F32, kind="ExternalOutput").ap()
        probe = None

    with tile.TileContext(nc) as tc:
        with tc.tile_pool(name="persist", bufs=1) as pp:
            V = [pp.tile([P, 768], F32R, name=f"v{j}", tag=f"v{j}") for j in range(NT)]
            mask_t = pp.tile([P, 4 * 512], F32, tag="mask_t")
            bq_t = [pp.tile([P, 1], F32, name=f"bq{m}", tag=f"bq{m}") for m in range(4)]
            bk_t = [pp.tile([P, 1], F32, name=f"bk{m}", tag=f"bk{m}") for m in range(4)]
            bv_t = pp.tile([1, 512], F32R, tag="bv")
            ones_t = pp.tile([1, P], F32R, tag="ones")

            nc.sync.dma_start(mask_t[:], mask[:])
            for m in range(4):
                nc.sync.dma_start(bq_t[m][:], bq[m * P:(m + 1) * P, :])
                nc.sync.dma_start(bk_t[m][:], bk[m * P:(m + 1) * P, :])
            nc.sync.dma_start(bv_t[:], bv[:].bitcast(F32R))
            nc.vector.memset(ones_t[:].bitcast(F32), 1.0)
            for j in range(NT):
                nc.vector.memset(V[j][:].bitcast(F32), 1.0)

            chain = repeat > 1
            tok = None
            for _rep in range(repeat):
                rep_pool_cm = tc.tile_pool(name=f"rp{_rep}", bufs=1)
                rp = rep_pool_cm.__enter__()
                if chain and _rep > 0:
                    # serialize iterations (timing builds): biases depend on
                    # 0 x previous iteration's output.
                    bq_e, bk_e = [], []
                    for m in range(4):
                        t1 = rp.tile([P, 1], F32, name=f"bqe{_rep}_{m}", tag=f"bqe{_rep}_{m}")
                        nc.vector.tensor_add(t1[:], bq_t[m][:], tok[:])
                        bq_e.append(t1)
                        t2 = rp.tile([P, 1], F32, name=f"bke{_rep}_{m}", tag=f"bke{_rep}_{m}")
                        nc.vector.tensor_add(t2[:], bk_t[m][:], tok[:])
                        bk_e.append(t2)
                    bv_e = rp.tile([1, 512], F32R, name=f"bve{_rep}", tag=f"bve{_rep}")
                    nc.vector.tensor_scalar_add(bv_e[:], bv_t[:], tok[0:1, 0:1])
                else:
                    bq_e, bk_e, bv_e = bq_t, bk_t, bv_t

                # ---- phase C: V projection (V = x @ Wv + bias row) ----
                with tc.tile_pool(name="wvt", bufs=1) as wvpool, \
                     tc.tile_pool(name="xtv", bufs=10) as xpool, \
                     tc.tile_pool(name="vps", bufs=6, space="PSUM") as vps:
                    WV = [wvpool.tile([P, 512], F32R, name=f"wv{k}", tag=f"wv{k}") for k in range(CK)]
                    for k in range(CK):
                        nc.sync.dma_start(WV[k][:], wv[k * P:(k + 1) * P, :].bitcast(F32R))
                    for n in range(4):
                        xts = []
                        for k in range(CK):
                            xtile = xpool.tile([P, 512], F32R, tag="xt")
                            nc.sync.dma_start(
                                xtile[:],
                                xt[k * P:(k + 1) * P, n * 512:(n + 1) * 512].bitcast(F32R),
                            )
                            xts.append(xtile)
                        for i in range(4):
                            ps = vps.tile([P, 512], F32, tag="ps")
                            for k in range(CK):
                                nc.tensor.matmul(
                                    ps[:], xts[k][:, i * P:(i + 1) * P], WV[k][:],
                                    start=(k == 0), stop=False,
                                )
                            nc.tensor.matmul(ps[:], ones_t[:], bv_e[:],
                                             start=False, stop=True)
                            vap = V[n * 4 + i][:]
                            part = list(vap.ap)[0]
                            psap = ps[:]
                            pspart = list(psap.ap)[0]
                            nc.vector.tensor_copy(
                                AP(vap.tensor, vap.offset, [part, (192, 4), (1, 64)]),
                                AP(psap.tensor, psap.offset, [pspart, (128, 4), (1, 64)]))
                            nc.vector.tensor_copy(
                                AP(vap.tensor, vap.offset + 128, [part, (192, 4), (1, 64)]),
                                AP(psap.tensor, psap.offset + 64, [pspart, (128, 4), (1, 64)]))

                # ---- interleaved B (Q/K projection per pair) and D ----
                bd = tc.tile_pool(name="bd", bufs=1)
                bdp = bd.__enter__()
                WQ = [bdp.tile([P, 512], F32R, name=f"wq{k}", tag=f"wq{k}") for k in range(CK)]
                WK = [bdp.tile([P, 512], F32R, name=f"wk{k}", tag=f"wk{k}") for k in range(CK)]
                for k in range(CK):
                    nc.sync.dma_start(WQ[k][:], wq[k * P:(k + 1) * P, :].bitcast(F32R))
                    nc.sync.dma_start(WK[k][:], wk[k * P:(k + 1) * P, :].bitcast(F32R))
                YT = [bdp.tile([P, T], F32R, name=f"yt{p}", tag=f"yt{p}") for p in range(4)]

                xqpool = tc.tile_pool(name="xtq", bufs=10)
                xqp = xqpool.__enter__()
                qtpool = tc.tile_pool(name="qtp", bufs=2)
                qtp = qtpool.__enter__()
                bdps = tc.tile_pool(name="bdps", bufs=2, space="PSUM")
                bdpsp = bdps.__enter__()
                spool_cm = tc.tile_pool(name="spsum", bufs=2, space="PSUM")
                spool = spool_cm.__enter__()
                avpool_cm = tc.tile_pool(name="avps", bufs=2, space="PSUM")
                avpool = avpool_cm.__enter__()
                ptpool_cm = tc.tile_pool(name="pt", bufs=2)
                ptpool = ptpool_cm.__enter__()
                lpool_cm = tc.tile_pool(name="lrec", bufs=1)
                lpool = lpool_cm.__enter__()

                QTs = [None] * 4
                KTs = [None] * 4

                def b_step(p, n):
                    # Q^T/K^T chunk for pair p, token-slice n
                    if n == 0:
                        QTs[p] = qtp.tile([P, T], F32R, name=f"qtt{p}", tag="qtt")
                        KTs[p] = qtp.tile([P, T], F32R, name=f"ktt{p}", tag="ktt")
                    xts = []
                    for k in range(CK):
                        xtile = xqp.tile([P, 512], F32R, tag="xt2")
                        nc.sync.dma_start(
                            xtile[:],
                            xt[k * P:(k + 1) * P, n * 512:(n + 1) * 512].bitcast(F32R),
                        )
                        xts.append(xtile)
                    ps = bdpsp.tile([P, 512], F32, tag="bps")
                    for k in range(CK):
                        nc.tensor.matmul(
                            ps[:], WQ[k][:, p * P:(p + 1) * P], xts[k][:],
                            start=(k == 0), stop=(k == CK - 1),
                        )
                    nc.vector.tensor_scalar_add(
                        QTs[p][:, n * 512:(n + 1) * 512], ps[:], bq_e[p][:])
                    ps = bdpsp.tile([P, 512], F32, tag="bps")
                    for k in range(CK):
                        nc.tensor.matmul(
                            ps[:], WK[k][:, p * P:(p + 1) * P], xts[k][:],
                            start=(k == 0), stop=(k == CK - 1),
                        )
                    nc.vector.tensor_scalar_add(
                        KTs[p][:, n * 512:(n + 1) * 512], ps[:], bk_e[p][:])

                def v_stat_ap(j: int, h: int) -> AP:
                    # [V_even|ones|V_odd] per pair: even h -> [V_h | ones]
                    # (O^T rows 0:64, denom rows 64:128); odd h -> [ones | V_h].
                    e = h // 2
                    start = 192 * e + (64 if h % 2 else 0)
                    return V[j][:, start:start + 128]

                def d_group(p, hh, s):
                    h = 2 * p + hh
                    prow = hh * 64       # O^T partition rows
                    lrow = 64 - prow     # denominator partition rows
                    QT, KT = QTs[p], KTs[p]
                    nch = 4 * s + 4
                    npairs = nch // 2
                    av = avpool.tile([P, 512], F32, tag="av")
                    pts = []

                    def emit_s(jj):
                        sp = spool.tile([P, 1024], F32, tag="sp")
                        for cc in range(2):
                            j = 2 * jj + cc
                            nc.tensor.matmul(
                                sp[:, cc * 512:(cc + 1) * 512],
                                KT[prow:prow + 64, j * P:(j + 1) * P],
                                QT[prow:prow + 64, s * 512:(s + 1) * 512],
                                start=True, stop=True,
                            )
                            r = j - 4 * s
                            if 0 <= r <= 3:
                                # one additive mask covering the whole
                                # invalid region of this chunk
                                w = (r + 1) * P
                                nc.vector.tensor_add(
                                    sp[:, cc * 512: cc * 512 + w],
                                    sp[:, cc * 512: cc * 512 + w],
                                    mask_t[:, r * 512: r * 512 + w],
                                )
                        pt = ptpool.tile([P, 1024], F32R, tag="pt")
                        nc.scalar.activation(pt[:], sp[:], AF.Exp, scale=0.125)
                        pts.append(pt)

                    def emit_av(jj):
                        pt = pts[jj]
                        for cc in range(2):
                            j = 2 * jj + cc
                            nc.tensor.matmul(
                                av[:], v_stat_ap(j, h),
                                pt[:, cc * 512:(cc + 1) * 512],
                                start=(j == 0), stop=(j == nch - 1),
                            )

                    emit_s(0)
                    for jj in range(1, npairs):
                        emit_s(jj)
                        emit_av(jj - 1)
                    emit_av(npairs - 1)

                    # normalization: denom rows -> SBUF (same partitions),
                    # DMA-shift to O^T's partitions, reciprocal + multiply.
                    lt = lpool.tile([P, 512], F32, tag="lt")
                    nc.vector.tensor_copy(
                        lt[lrow:lrow + 64, :], av[lrow:lrow + 64, :])
                    lt2 = lpool.tile([P, 512], F32, tag="lt2")
                    nc.sync.dma_start(
                        lt2[prow:prow + 64, :], lt[lrow:lrow + 64, :])
                    rec = lpool.tile([P, 512], F32, tag="rec")
                    nc.vector.reciprocal(
                        rec[prow:prow + 64, :], lt2[prow:prow + 64, :])
                    nc.vector.tensor_mul(
                        YT[p][prow:prow + 64, s * 512:(s + 1) * 512],
                        av[prow:prow + 64, :],
                        rec[prow:prow + 64, :],
                    )

                # emission: B(0) fully, then for each pair interleave its 8
                # attention groups with the next pair's 4 projection steps.
                for n in range(4):
                    b_step(0, n)
                for p in range(4):
                    groups = [(p, hh, s) for hh in range(2) for s in range(NS)]
                    bsteps = [(p + 1, n) for n in range(4)] if p < 3 else []
                    gi = 0
                    for i, g in enumerate(groups):
                        d_group(*g)
                        if i % 2 == 1 and gi < len(bsteps):
                            b_step(*bsteps[gi])
                            gi += 1
                    while gi < len(bsteps):
                        b_step(*bsteps[gi])
                        gi += 1

                lpool_cm.__exit__(None, None, None)
                ptpool_cm.__exit__(None, None, None)
                avpool_cm.__exit__(None, None, None)
                spool_cm.__exit__(None, None, None)
                bdps.__exit__(None, None, None)
                qtpool.__exit__(None, None, None)
                xqpool.__exit__(None, None, None)

                # ---- phase E: output projection ----
                WP = [bdp.tile([P, C], F32R, name=f"wp{k}", tag=f"wp{k}") for k in range(4)]
                for k in range(4):
                    nc.sync.dma_start(WP[k][:], wp[k * P:(k + 1) * P, :].bitcast(F32R))
                with tc.tile_pool(name="projps", bufs=4, space="PSUM") as prpool, \
                     tc.tile_pool(name="ostage", bufs=4) as opool:
                    for m in range(NT):
                        for nn in range(2):
                            ps = prpool.tile([P, 512], F32, tag="pp")
                            for kf in range(4):
                                nc.tensor.matmul(
                                    ps[:],
                                    YT[kf][:, m * P:(m + 1) * P],
                                    WP[kf][:, nn * 512:(nn + 1) * 512],
                                    start=(kf == 0), stop=(kf == 3),
                                )
                            ost = opool.tile([P, 512], F32, tag="ost")
                            nc.vector.tensor_copy(ost[:], ps[:])
                            nc.sync.dma_start(
                                out[m * P:(m + 1) * P, nn * 512:(nn + 1) * 512], ost[:])
                            last_ost = ost
                    if chain:
                        tok = pp.tile([P, 1], F32, name=f"tok{_rep}", tag=f"tok{_rep}")
                        nc.vector.tensor_scalar_mul(tok[:], last_ost[:, 0:1], 0.0)
                    if timing and _rep == repeat - 1:
                        nc.sync.dma_start(probe[:], last_ost[:, 0:4])
                bd.__exit__(None, None, None)
                rep_pool_cm.__exit__(None, None, None)

    nc.compile()
    return nc


_TRI = np.where(
    np.arange(P)[:, None] > np.arange(P)[None, :],
    np.float32(NEG), np.float32(0.0),
).astype(np.float32)


def _build_mask():
    m = np.zeros((P, 4 * 512), dtype=np.float32)
    for r in range(4):
        blk = m[:, r * 512:(r + 1) * 512]
        blk[:, : r * P] = np.float32(NEG)       # fully-masked col blocks
        blk[:, r * P:(r + 1) * P] = _TRI        # diagonal triangle
    return m


_MASK = _build_mask()


def shard_inputs(x, w_attn, b_attn, w_proj, b_proj):
    """Build the per-core input maps."""
    x = np.asarray(x, dtype=np.float32)
    w_attn = np.asarray(w_attn, dtype=np.float32)
    b_attn = np.asarray(b_attn, dtype=np.float32)
    w_proj = np.asarray(w_proj, dtype=np.float32)
    in_maps = []
    for c in range(N_CORES):
        b, g = divmod(c, 2)
        sl = slice(g * 512, (g + 1) * 512)
        in_maps.append({
            "xt": np.ascontiguousarray(x[b].T),
            "wq": np.ascontiguousarray(w_attn[:, g * 512:(g + 1) * 512]),
            "wk": np.ascontiguousarray(w_attn[:, 1024 + g * 512:1024 + (g + 1) * 512]),
            "wv": np.ascontiguousarray(w_attn[:, 2048 + g * 512:2048 + (g + 1) * 512]),
            "wp": np.ascontiguousarray(w_proj[g * 512:(g + 1) * 512, :]),
            "bq": np.ascontiguousarray(b_attn[sl].reshape(512, 1)),
            "bk": np.ascontiguousarray(b_attn[1024 + g * 512:1024 + (g + 1) * 512].reshape(512, 1)),
            "bv": np.ascontiguousarray(b_attn[2048 + g * 512:2048 + (g + 1) * 512].reshape(1, 512)),
            "mask": _MASK,
        })
    return in_maps


def gather_output(results, b_proj):
    b_proj = np.asarray(b_proj, dtype=np.float32)
    out = np.empty((4, T, C), dtype=np.float32)
    for b in range(4):
        out[b] = results[2 * b]["out"] + results[2 * b + 1]["out"] + b_proj
    return out


_NC_CACHE = None


def get_nc():
    global _NC_CACHE
    if _NC_CACHE is None:
        _NC_CACHE = build_nc()
    return _NC_CACHE


def kernel(x, w_attn, b_attn, w_proj, b_proj):
    from concourse.bass_utils import run_bass_kernel_spmd

    in_maps = shard_inputs(x, w_attn, b_attn, w_proj, b_proj)
    nc = get_nc()
    res = run_bass_kernel_spmd(nc, in_maps, list(range(N_CORES)))
    return gather_output(res.results, b_proj)



# revision 27
# speedup vs baseline: 2.5310x; 2.5310x over previous
"""Causal self-attention on 8 Trainium2 NeuronCores.

Sharding: core c = 2*b + g handles batch b (of 4) and head-group g (8 of 16
heads). Each core computes the qkv projection for its head slice, causal
attention for its 8 heads, and the output projection against its 512-row
slice of w_proj. The two half-projections per batch are summed on the host
(plus b_proj).

Per-core structure (v2):
  x^T is DMA'd ONCE into SBUF (8 chunks [128, 2048] f32) and reused by the
  V / Q / K projections (the baseline re-streamed x from HBM per pair: 40MB
  -> 8MB of DMA). DMA traffic is split across the SP, Activation and Pool
  hw queues.
  Q^T/K^T are stored bf16 per pair of heads; V token-major bf16 as
  [V_even | ones | V_odd] per pair (the `ones` columns give the softmax
  denominator for free inside the AV matmul). exp() output (pt) is bf16.
  Causal masking is fine-grained: for the diagonal 512-query super-block,
  the S^T matmuls only compute the live (key <= query) columns, in 512/384/
  256/128-wide slabs, with a single additive [128,128] triangle mask per
  diagonal 128-block. This removes ~20% of S/exp/AV work vs 512-granular
  masking.
  PSUM->SBUF staging copies run on the (otherwise idle) GpSimd engine.
All matmuls run in float32r / bf16 (~2e-3 max rel err).
"""

import sys

sys.path.insert(0, "/opt/trn_rl_repo")

import numpy as np

import concourse.bass as bass
import concourse.mybir as mybir
import concourse.tile as tile
from concourse import bacc
from concourse.bass import AP

F32 = mybir.dt.float32
F32R = mybir.dt.float32r
BF16 = mybir.dt.bfloat16
FP8 = mybir.dt.float8e4
DOUBLE_ROW = mybir.MatmulPerfMode.DoubleRow
AF = mybir.ActivationFunctionType

N_CORES = 8
T = 2048
C = 1024
D = 64
P = 128
NT = T // P     # 16 token chunks
NS = 4          # q super-blocks of 512
CK = C // P     # 8 contraction chunks
NEG = -1e9


def build_nc(repeat: int = 1, timing: bool = False):
    nc = bacc.Bacc("TRN2", target_bir_lowering=False, debug=False)

    xt = nc.dram_tensor("xt", [C, T], F32, kind="ExternalInput").ap()
    wq = nc.dram_tensor("wq", [C, 512], F32, kind="ExternalInput").ap()
    wk = nc.dram_tensor("wk", [C, 512], F32, kind="ExternalInput").ap()
    wv = nc.dram_tensor("wv", [C, 512], F32, kind="ExternalInput").ap()
    wp = nc.dram_tensor("wp", [512, C], F32, kind="ExternalInput").ap()
    bq = nc.dram_tensor("bq", [512, 1], F32, kind="ExternalInput").ap()
    bk = nc.dram_tensor("bk", [512, 1], F32, kind="ExternalInput").ap()
    bv = nc.dram_tensor("bv", [1, 512], F32, kind="ExternalInput").ap()
    # trib = transpose of the additive causal mask for a diagonal 128-block
    # (strictly-lower triangle of S^T = NEG): used as the stationary of a
    # tiny accumulate-matmul (trib^T @ I) that applies the mask on the PE,
    # keeping the S -> exp chain off the DVE.
    trib = nc.dram_tensor("trib", [P, P], F32, kind="ExternalInput").ap()
    eye = nc.dram_tensor("eye", [P, P], F32, kind="ExternalInput").ap()
    if timing:
        out = nc.dram_tensor("out", [T, C], F32).ap()
        probe = nc.dram_tensor("probe", [P, 4], F32, kind="ExternalOutput").ap()
    else:
        out = nc.dram_tensor("out", [T, C], F32, kind="ExternalOutput").ap()
        probe = None

    with tile.TileContext(nc) as tc:
        with tc.tile_pool(name="persist", bufs=1) as pp:
            V = [pp.tile([P, 768], BF16, name=f"v{j}", tag=f"v{j}") for j in range(NT)]
            YT = [pp.tile([P, T], BF16, name=f"yt{p}", tag=f"yt{p}") for p in range(4)]
            trib_f = pp.tile([P, P], F32, tag="trib_f")
            trib_t = pp.tile([P, P], BF16, tag="trib_t")
            eye_f = pp.tile([P, P], F32, tag="eye_f")
            eye_t = pp.tile([P, P], BF16, tag="eye_t")
            bq_t = [pp.tile([P, 1], F32, name=f"bq{m}", tag=f"bq{m}") for m in range(4)]
            bk_t = [pp.tile([P, 1], F32, name=f"bk{m}", tag=f"bk{m}") for m in range(4)]
            bv_t = pp.tile([1, 512], F32R, tag="bv")
            ones_t = pp.tile([1, P], F32R, tag="ones")

            nc.gpsimd.dma_start(trib_f[:], trib[:])
            nc.gpsimd.dma_start(eye_f[:], eye[:])
            nc.vector.tensor_copy(trib_t[:], trib_f[:])
            nc.vector.tensor_copy(eye_t[:], eye_f[:])
            nc.vector.memset(ones_t[:].bitcast(F32), 1.0)
            # only the `ones` columns [64:128) of each 192-col pair block need
            # 1.0 -- the V halves are overwritten every iteration.
            for j in range(NT):
                vap = V[j][:]
                part = list(vap.ap)[0]
                nc.vector.memset(
                    AP(vap.tensor, vap.offset + 64, [part, (192, 4), (1, 64)]), 1.0)

            chain = repeat > 1
            tok = None
            for _rep in range(repeat):
                rep_pool_cm = tc.tile_pool(name=f"rp{_rep}", bufs=1)
                rp = rep_pool_cm.__enter__()
                if chain and _rep > 0:
                    # serialize iterations (timing builds): biases depend on
                    # 0 x previous iteration's output.
                    bq_e, bk_e = [], []
                    for m in range(4):
                        t1 = rp.tile([P, 1], F32, name=f"bqe{_rep}_{m}", tag=f"bqe{_rep}_{m}")
                        nc.vector.tensor_add(t1[:], bq_t[m][:], tok[:])
                        bq_e.append(t1)
                        t2 = rp.tile([P, 1], F32, name=f"bke{_rep}_{m}", tag=f"bke{_rep}_{m}")
                        nc.vector.tensor_add(t2[:], bk_t[m][:], tok[:])
                        bk_e.append(t2)
                    bv_e = rp.tile([1, 512], F32R, name=f"bve{_rep}", tag=f"bve{_rep}")
                    nc.vector.tensor_scalar_add(bv_e[:], bv_t[:], tok[0:1, 0:1])
                else:
                    bq_e, bk_e, bv_e = bq_t, bk_t, bv_t

                xpool_cm = tc.tile_pool(name=f"xp{_rep}", bufs=1)
                xp = xpool_cm.__enter__()
                X = [xp.tile([P, T], F32R, name=f"x{k}", tag=f"x{k}") for k in range(CK)]

                # wq/wk tiles allocated BEFORE wv so their space doesn't alias
                # the (still-live) WV tiles and their DMAs can stream during
                # the V phase.
                bd = tc.tile_pool(name="bd", bufs=1)
                bdp = bd.__enter__()
                WQ = [bdp.tile([P, 512], F32R, name=f"wq{k}", tag=f"wq{k}") for k in range(CK)]
                WK = [bdp.tile([P, 512], F32R, name=f"wk{k}", tag=f"wk{k}") for k in range(CK)]

                # ---- phase C: V projection (V = x @ Wv + bias row) ----
                with tc.tile_pool(name="wvt", bufs=1) as wvpool, \
                     tc.tile_pool(name="vps", bufs=6, space="PSUM") as vps:
                    WV = [wvpool.tile([P, 512], F32R, name=f"wv{k}", tag=f"wv{k}") for k in range(CK)]
                    # first-needed tensors (x slab 0 + wv) split 3-way across
                    # the SP/ACT/Pool queues -> V compute starts ~5us in
                    for k in range(6):
                        nc.sync.dma_start(
                            X[k][:, 0:512], xt[k * P:(k + 1) * P, 0:512].bitcast(F32R))
                    for k in range(6):
                        nc.scalar.dma_start(WV[k][:], wv[k * P:(k + 1) * P, :].bitcast(F32R))
                    for k in range(6, CK):
                        nc.gpsimd.dma_start(
                            X[k][:, 0:512], xt[k * P:(k + 1) * P, 0:512].bitcast(F32R))
                    for k in range(6, CK):
                        nc.gpsimd.dma_start(WV[k][:], wv[k * P:(k + 1) * P, :].bitcast(F32R))
                    nc.sync.dma_start(bv_t[:], bv[:].bitcast(F32R))
                    for k in range(4):
                        nc.sync.dma_start(
                            X[k][:, 512:1024], xt[k * P:(k + 1) * P, 512:1024].bitcast(F32R))
                    for k in range(4, CK):
                        nc.scalar.dma_start(
                            X[k][:, 512:1024], xt[k * P:(k + 1) * P, 512:1024].bitcast(F32R))
                    for m in range(4):
                        nc.sync.dma_start(bq_t[m][:], bq[m * P:(m + 1) * P, :])
                        nc.sync.dma_start(bk_t[m][:], bk[m * P:(m + 1) * P, :])
                    for k in range(CK):
                        nc.scalar.dma_start(
                            X[k][:, 1024:1536], xt[k * P:(k + 1) * P, 1024:1536].bitcast(F32R))
                    for k in range(CK):
                        nc.sync.dma_start(
                            X[k][:, 1536:2048], xt[k * P:(k + 1) * P, 1536:2048].bitcast(F32R))
                    for k in range(CK):
                        nc.sync.dma_start(WQ[k][:], wq[k * P:(k + 1) * P, :].bitcast(F32R))
                        nc.scalar.dma_start(WK[k][:], wk[k * P:(k + 1) * P, :].bitcast(F32R))

                    for i in range(NT):
                        ps = vps.tile([P, 512], F32, tag="ps")
                        for k in range(CK):
                            nc.tensor.matmul(
                                ps[:], X[k][:, i * P:(i + 1) * P], WV[k][:],
                                start=(k == 0), stop=False,
                            )
                        nc.tensor.matmul(ps[:], ones_t[:], bv_e[:],
                                         start=False, stop=True)
                        # interleave into [V_e | ones | V_o] (bf16)
                        vap = V[i][:]
                        part = list(vap.ap)[0]
                        psap = ps[:]
                        pspart = list(psap.ap)[0]
                        nc.vector.tensor_copy(
                            AP(vap.tensor, vap.offset, [part, (192, 4), (1, 64)]),
                            AP(psap.tensor, psap.offset, [pspart, (128, 4), (1, 64)]))
                        nc.vector.tensor_copy(
                            AP(vap.tensor, vap.offset + 128, [part, (192, 4), (1, 64)]),
                            AP(psap.tensor, psap.offset + 64, [pspart, (128, 4), (1, 64)]))

                # ---- interleaved B (Q/K projection per pair) and D ----
                qtpool = tc.tile_pool(name="qtp", bufs=2)
                qtp = qtpool.__enter__()
                bdps = tc.tile_pool(name="bdps", bufs=2, space="PSUM")
                bdpsp = bdps.__enter__()
                lpool_cm = tc.tile_pool(name="lrec", bufs=2)
                lpool = lpool_cm.__enter__()
                opool_cm = tc.tile_pool(name="ostage", bufs=4)
                opool = opool_cm.__enter__()
                # attention-only pools last, so they can close before the
                # output-projection tail frees their PSUM banks
                spool_cm = tc.tile_pool(name="spsum", bufs=2, space="PSUM")
                spool = spool_cm.__enter__()
                avpool_cm = tc.tile_pool(name="avps", bufs=2, space="PSUM")
                avpool = avpool_cm.__enter__()
                ptpool_cm = tc.tile_pool(name="pt", bufs=2)
                ptpool = ptpool_cm.__enter__()

                QTs = [None] * 4
                KTs = [None] * 4
                QPs = [None] * 4
                KPs = [None] * 4

                def pack(dst, src_t):
                    # [128, T] fp8 -> [64, 2, T] double-row packed, one DMA
                    dap = dst[:]
                    part = list(dap.ap)[0]
                    nc.sync.dma_start(
                        AP(dap.tensor, dap.offset, [part, (T, 2), (1, T)]),
                        src_t[:])

                def b_step_micro(p, n):
                    # Q^T/K^T chunk for pair p, token-slice n (x from SBUF),
                    # written as fp8e4 and double-row packed after slab 3 for
                    # the half-rate S matmuls. Returned as 8 micro-steps of 2
                    # matmuls (~0.4us) each, to be interleaved between
                    # attention units (which are exp-paced, not PE-paced).
                    cell = {}

                    def mk(which, kk):
                        W, QK, QKP, b_e = (
                            (WQ, QTs, QPs, bq_e) if which == "q"
                            else (WK, KTs, KPs, bk_e))

                        def f():
                            if which == "q" and kk == 0 and n == 0:
                                QTs[p] = qtp.tile([P, T], FP8, name=f"qtt{p}", tag="qtt")
                                KTs[p] = qtp.tile([P, T], FP8, name=f"ktt{p}", tag="ktt")
                                QPs[p] = qtp.tile([64, 2 * T], FP8, name=f"qpp{p}", tag="qpp")
                                KPs[p] = qtp.tile([64, 2 * T], FP8, name=f"kpp{p}", tag="kpp")
                            if kk == 0:
                                cell[which] = bdpsp.tile([P, 512], F32, name="bps", tag="bps")
                            ps = cell[which]
                            for k in (2 * kk, 2 * kk + 1):
                                nc.tensor.matmul(
                                    ps[:], W[k][:, p * P:(p + 1) * P],
                                    X[k][:, n * 512:(n + 1) * 512],
                                    start=(k == 0), stop=(k == CK - 1),
                                )
                            if kk == 3:
                                nc.vector.tensor_scalar_add(
                                    QK[p][:, n * 512:(n + 1) * 512], ps[:], b_e[p][:])
                                if n == 3:
                                    pack(QKP[p], QK[p])
                        return f

                    return [mk(w, kk) for w in ("q", "k") for kk in range(4)]

                def v_stat_ap(j: int, h: int) -> AP:
                    # [V_even|ones|V_odd] per pair: even h -> [V_h | ones]
                    # (O^T rows 0:64, denom rows 64:128); odd h -> [ones | V_h].
                    e = h // 2
                    start = 192 * e + (64 if h % 2 else 0)
                    return V[j][:, start:start + 128]

                def d_group(p, s):
                    # fused pipeline over both heads hh=0,1 of the pair:
                    # units of the two heads alternate, doubling the
                    # independent work between an S matmul and the exp it
                    # feeds (the exp is slower than the matmuls it paces).
                    QP, KP = QPs[p], KPs[p]
                    av = {hh: avpool.tile([P, 512], F32, name=f"av{hh}", tag="av")
                          for hh in range(2)}

                    def s_mm(sp, spc0, hh, j, q0, w, start=True, stop=True):
                        # S^T chunk [128 keys, w queries] via fp8 DoubleRow
                        qb = QP[hh * 32:(hh + 1) * 32, :]
                        kb = KP[hh * 32:(hh + 1) * 32, :]
                        qpart = list(qb.ap)[0]
                        kpart = list(kb.ap)[0]
                        nc.tensor.matmul(
                            sp[:, spc0:spc0 + w],
                            AP(kb.tensor, kb.offset + j * P, [kpart, (T, 2), (1, P)]),
                            AP(qb.tensor, qb.offset + q0, [qpart, (T, 2), (1, w)]),
                            start=start, stop=stop, perf_mode=DOUBLE_ROW,
                        )

                    def tri_mm(sp, spc0, stop):
                        # accumulate the NEG triangle into sp[:, spc0:+128]
                        # via trib^T @ I on the PE (128 cycles)
                        nc.tensor.matmul(
                            sp[:, spc0:spc0 + P], trib_t[:], eye_t[:],
                            start=False, stop=stop,
                        )

                    # units: [full pairs of chunks...] + [diagA, diagB]
                    # full unit jj: chunks 2jj, 2jj+1 (full 512-wide)
                    # diagA: j=4s   @ sp[0:512]  (512w), j=4s+1 @ sp[512:896] (384w)
                    # diagB: j=4s+2 @ sp[0:256]  (256w), j=4s+3 @ sp[256:384] (128w)
                    units = [("full", jj) for jj in range(2 * s)] + [("dA", 0), ("dB", 0)]
                    pts = {0: [], 1: []}

                    def emit_s(hh, u):
                        kind, jj = units[u]
                        sp = spool.tile([P, 1024], F32, tag="sp")
                        if kind == "full":
                            for cc in range(2):
                                j = 2 * jj + cc
                                s_mm(sp, cc * 512, hh, j, s * 512, 512)
                            ew = 1024
                        elif kind == "dA":
                            j0 = 4 * s
                            s_mm(sp, 0, hh, j0, s * 512, 512, stop=False)
                            s_mm(sp, 512, hh, j0 + 1, s * 512 + 128, 384, stop=False)
                            tri_mm(sp, 0, stop=True)      # bank 0
                            tri_mm(sp, 512, stop=True)    # bank 1
                            ew = 896
                        else:  # dB (both chunks + both triangles in bank 0)
                            j2 = 4 * s + 2
                            s_mm(sp, 0, hh, j2, s * 512 + 256, 256, stop=False)
                            s_mm(sp, 256, hh, j2 + 1, s * 512 + 384, 128,
                                 start=False, stop=False)
                            tri_mm(sp, 0, stop=False)
                            tri_mm(sp, 256, stop=True)
                            ew = 384
                        pt = ptpool.tile([P, 1024], BF16, tag="pt")
                        nc.scalar.activation(pt[:, 0:ew], sp[:, 0:ew], AF.Exp, scale=0.125)
                        pts[hh].append(pt)

                    def emit_av(hh, u):
                        h = 2 * p + hh
                        kind, jj = units[u]
                        pt = pts[hh][u]
                        avh = av[hh]
                        if kind == "full":
                            for cc in range(2):
                                j = 2 * jj + cc
                                nc.tensor.matmul(
                                    avh[:], v_stat_ap(j, h),
                                    pt[:, cc * 512:(cc + 1) * 512],
                                    start=(j == 0), stop=False,
                                )
                        elif kind == "dA":
                            j0 = 4 * s
                            nc.tensor.matmul(
                                avh[:], v_stat_ap(j0, h), pt[:, 0:512],
                                start=(j0 == 0), stop=False,
                            )
                            nc.tensor.matmul(
                                avh[:, 128:512], v_stat_ap(j0 + 1, h), pt[:, 512:896],
                                start=False, stop=False,
                            )
                        else:  # dB
                            j2 = 4 * s + 2
                            nc.tensor.matmul(
                                avh[:, 256:512], v_stat_ap(j2, h), pt[:, 0:256],
                                start=False, stop=False,
                            )
                            nc.tensor.matmul(
                                avh[:, 384:512], v_stat_ap(j2 + 1, h), pt[:, 256:384],
                                start=False, stop=True,
                            )

                    def finish(hh):
                        # evict av to SBUF at once (frees the PSUM slot for
                        # the next group's AV), then normalize off the
                        # critical path: reciprocal of the denominator rows,
                        # DMA-shift to O^T's partitions, multiply into YT.
                        prow = hh * 64
                        lrow = 64 - prow
                        avs = lpool.tile([P, 512], F32, tag="avs")
                        nc.vector.tensor_copy(avs[:], av[hh][:])
                        rec = lpool.tile([P, 512], F32, tag="rec")
                        nc.vector.reciprocal(
                            rec[lrow:lrow + 64, :], avs[lrow:lrow + 64, :])
                        rec2 = lpool.tile([P, 512], F32, tag="rec2")
                        nc.gpsimd.dma_start(
                            rec2[prow:prow + 64, :], rec[lrow:lrow + 64, :])
                        nc.vector.tensor_mul(
                            YT[p][prow:prow + 64, s * 512:(s + 1) * 512],
                            avs[prow:prow + 64, :],
                            rec2[prow:prow + 64, :],
                        )

                    nu = len(units)
                    seq = [(hh, u) for u in range(nu) for hh in range(2)]
                    emit_s(*seq[0])
                    yield
                    emit_s(*seq[1])
                    yield
                    for i in range(2, len(seq)):
                        emit_s(*seq[i])
                        emit_av(*seq[i - 2])
                        yield
                    emit_av(*seq[-2])
                    emit_av(*seq[-1])
                    finish(0)
                    finish(1)

                # ---- phase E: output projection, interleaved into pair-3's
                # attention (its E blocks pad the ACT-paced attention the way
                # b_steps pad pairs 0-2). Uses the b_step PSUM tiles.
                WPb = [bdp.tile([P, C], BF16, name=f"wpb{k}", tag=f"wpb{k}")
                       for k in range(4)]
                last_ost = [None]

                def load_wp():
                    for k in range(4):
                        for h2 in range(2):
                            wpf = lpool.tile([P, 512], F32, name=f"wpf{k}_{h2}", tag="wpf")
                            nc.sync.dma_start(
                                wpf[:], wp[k * P:(k + 1) * P, h2 * 512:(h2 + 1) * 512])
                            nc.gpsimd.tensor_copy(
                                WPb[k][:, h2 * 512:(h2 + 1) * 512], wpf[:])

                def e_step(m, nn, pspool, pstag, dma_eng=None):
                    def f():
                        ps = pspool.tile([P, 512], F32, tag=pstag)
                        for kf in range(4):
                            nc.tensor.matmul(
                                ps[:],
                                YT[kf][:, m * P:(m + 1) * P],
                                WPb[kf][:, nn * 512:(nn + 1) * 512],
                                start=(kf == 0), stop=(kf == 3),
                            )
                        ost = opool.tile([P, 512], F32, tag="ost")
                        nc.vector.tensor_copy(ost[:], ps[:])
                        (dma_eng or nc.sync).dma_start(
                            out[m * P:(m + 1) * P, nn * 512:(nn + 1) * 512], ost[:])
                        last_ost[0] = ost
                    return f

                # emission: B(0) fully, then per pair: fused groups in
                # descending-s order with the NEXT pair's projection
                # micro-steps (and, for pair 3, the output projection)
                # interleaved one per attention unit -- the units are
                # exp-paced so the PE has ~0.4us of slack per unit.
                from collections import deque
                pads = deque()

                def run_group(p, s):
                    for _ in d_group(p, s):
                        if pads:
                            pads.popleft()()

                for n in range(4):
                    for f in b_step_micro(0, n):
                        f()
                for p in range(3):
                    for n in range(4):
                        pads.extend(b_step_micro(p + 1, n))
                    for s in (3, 2, 1, 0):
                        run_group(p, s)
                    while pads:
                        pads.popleft()()
                load_wp()
                for s in (3, 2, 1, 0):
                    run_group(3, s)
                    for m in range(4 * s + 3, 4 * s - 1, -1):
                        if s > 0:
                            pads.append(e_step(m, 0, bdpsp, "bps"))
                            pads.append(e_step(m, 1, bdpsp, "bps"))
                while pads:
                    pads.popleft()()

                # tail: close the attention PSUM pools, run the last output
                # blocks with a 4-deep PSUM pipeline in the freed banks.
                ptpool_cm.__exit__(None, None, None)
                avpool_cm.__exit__(None, None, None)
                spool_cm.__exit__(None, None, None)
                with tc.tile_pool(name="projps", bufs=4, space="PSUM") as prpool:
                    for m in (3, 2, 1, 0):
                        e_step(m, 0, prpool, "pp", nc.sync)()
                        e_step(m, 1, prpool, "pp", nc.scalar)()

                    if chain:
                        tok = pp.tile([P, 1], F32, name=f"tok{_rep}", tag=f"tok{_rep}")
                        nc.vector.tensor_scalar_mul(tok[:], last_ost[0][:, 0:1], 0.0)
                    if timing and _rep == repeat - 1:
                        nc.sync.dma_start(probe[:], last_ost[0][:, 0:4])

                opool_cm.__exit__(None, None, None)
                lpool_cm.__exit__(None, None, None)
                bdps.__exit__(None, None, None)
                qtpool.__exit__(None, None, None)
                bd.__exit__(None, None, None)
                xpool_cm.__exit__(None, None, None)
                rep_pool_cm.__exit__(None, None, None)

    nc.compile()
    return nc


_TRI = np.where(
    np.arange(P)[:, None] > np.arange(P)[None, :],
    np.float32(NEG), np.float32(0.0),
).astype(np.float32)
_EYE = np.eye(P, dtype=np.float32)


def shard_inputs(x, w_attn, b_attn, w_proj, b_proj):
    """Build the per-core input maps."""
    x = np.asarray(x, dtype=np.float32)
    w_attn = np.asarray(w_attn, dtype=np.float32)
    b_attn = np.asarray(b_attn, dtype=np.float32)
    w_proj = np.asarray(w_proj, dtype=np.float32)
    in_maps = []
    for c in range(N_CORES):
        b, g = divmod(c, 2)
        sl = slice(g * 512, (g + 1) * 512)
        in_maps.append({
            "xt": np.ascontiguousarray(x[b].T),
            "wq": np.ascontiguousarray(w_attn[:, g * 512:(g + 1) * 512]),
            "wk": np.ascontiguousarray(w_attn[:, 1024 + g * 512:1024 + (g + 1) * 512]),
            "wv": np.ascontiguousarray(w_attn[:, 2048 + g * 512:2048 + (g + 1) * 512]),
            "wp": np.ascontiguousarray(w_proj[g * 512:(g + 1) * 512, :]),
            "bq": np.ascontiguousarray(b_attn[sl].reshape(512, 1)),
            "bk": np.ascontiguousarray(b_attn[1024 + g * 512:1024 + (g + 1) * 512].reshape(512, 1)),
            "bv": np.ascontiguousarray(b_attn[2048 + g * 512:2048 + (g + 1) * 512].reshape(1, 512)),
            "trib": np.ascontiguousarray(_TRI.T),
            "eye": _EYE,
        })
    return in_maps


def gather_output(results, b_proj):
    b_proj = np.asarray(b_proj, dtype=np.float32)
    out = np.empty((4, T, C), dtype=np.float32)
    for b in range(4):
        out[b] = results[2 * b]["out"] + results[2 * b + 1]["out"] + b_proj
    return out


_NC_CACHE = None


def get_nc():
    global _NC_CACHE
    if _NC_CACHE is None:
        _NC_CACHE = build_nc()
    return _NC_CACHE


def kernel(x, w_attn, b_attn, w_proj, b_proj):
    from concourse.bass_utils import run_bass_kernel_spmd

    in_maps = shard_inputs(x, w_attn, b_attn, w_proj, b_proj)
    nc = get_nc()
    res = run_bass_kernel_spmd(nc, in_maps, list(range(N_CORES)))
    return gather_output(res.results, b_proj)


# revision 29
# speedup vs baseline: 3.4959x; 1.3812x over previous
"""Causal self-attention on 8 Trainium2 NeuronCores.

Sharding: core c = 2*b + g handles batch b (of 4) and head-group g (8 of 16
heads). Each core computes the qkv projection for its head slice, causal
attention for its 8 heads, and the output projection against its 512-row
slice of w_proj. The two half-projections per batch are summed on the host
(plus b_proj).

Per-core structure (v3):
  x^T is DMA'd ONCE into SBUF (8 chunks [128, 2048] f32) and reused by the
  V / Q / K projections; DMA traffic is split across the SP / Activation /
  Pool hardware queues.
  Q^T/K^T are produced in fp8e4 and double-row packed so the S matmuls run
  at half cost (MatmulPerfMode.DoubleRow). V is token-major bf16 as
  [V_even | ones | V_odd] per pair (the `ones` columns produce the softmax
  denominator for free inside the AV matmul); exp() output is bf16.
  Causal masking is fine-grained: the diagonal 512-query super-block only
  computes live (key <= query) columns in 512/384/256/128-wide slabs; the
  128x128 triangle mask is applied ON THE PE as a tiny accumulate-matmul
  (trib^T @ I), keeping the S->exp chain off the DVE.
  Scheduling is a micro-pipeline: attention units (S -> exp -> AV) are
  exp-paced, so all other PE work -- the next pair's Q/K projection, the
  deferred V blocks 8..15, and the output projection -- is sliced into
  ~0.4us micro-steps and interleaved between units. Pair 0 runs its query
  super-blocks in ascending order so attention starts as soon as V[0..3]
  and its Q/K are ready (~determined by the first x slabs), ~15us earlier
  than building all of V first.
Max rel err ~9e-3 (fp8 logits; bf16 P/V; everything else f32/tf32).
"""

import sys

sys.path.insert(0, "/opt/trn_rl_repo")

import numpy as np

import concourse.bass as bass
import concourse.mybir as mybir
import concourse.tile as tile
from concourse import bacc
from concourse.bass import AP

F32 = mybir.dt.float32
F32R = mybir.dt.float32r
BF16 = mybir.dt.bfloat16
FP8 = mybir.dt.float8e4
DOUBLE_ROW = mybir.MatmulPerfMode.DoubleRow
AF = mybir.ActivationFunctionType

N_CORES = 8
T = 2048
C = 1024
D = 64
P = 128
NT = T // P     # 16 token chunks
NS = 4          # q super-blocks of 512
CK = C // P     # 8 contraction chunks
NEG = -1e9


def build_nc(repeat: int = 1, timing: bool = False):
    nc = bacc.Bacc("TRN2", target_bir_lowering=False, debug=False)

    xt = nc.dram_tensor("xt", [C, T], F32, kind="ExternalInput").ap()
    wq = nc.dram_tensor("wq", [C, 512], F32, kind="ExternalInput").ap()
    wk = nc.dram_tensor("wk", [C, 512], F32, kind="ExternalInput").ap()
    wv = nc.dram_tensor("wv", [C, 512], F32, kind="ExternalInput").ap()
    wp = nc.dram_tensor("wp", [512, C], F32, kind="ExternalInput").ap()
    bq = nc.dram_tensor("bq", [512, 1], F32, kind="ExternalInput").ap()
    bk = nc.dram_tensor("bk", [512, 1], F32, kind="ExternalInput").ap()
    bv = nc.dram_tensor("bv", [1, 512], F32, kind="ExternalInput").ap()
    # trib = transpose of the additive causal mask for a diagonal 128-block
    # (strictly-lower triangle of S^T = NEG): used as the stationary of a
    # tiny accumulate-matmul (trib^T @ I) that applies the mask on the PE.
    trib = nc.dram_tensor("trib", [P, P], F32, kind="ExternalInput").ap()
    eye = nc.dram_tensor("eye", [P, P], F32, kind="ExternalInput").ap()
    if timing:
        out = nc.dram_tensor("out", [T, C], F32).ap()
        probe = nc.dram_tensor("probe", [P, 4], F32, kind="ExternalOutput").ap()
    else:
        out = nc.dram_tensor("out", [T, C], F32, kind="ExternalOutput").ap()
        probe = None

    with tile.TileContext(nc) as tc:
        with tc.tile_pool(name="persist", bufs=1) as pp:
            V = [pp.tile([P, 768], BF16, name=f"v{j}", tag=f"v{j}") for j in range(NT)]
            YT = [pp.tile([P, T], BF16, name=f"yt{p}", tag=f"yt{p}") for p in range(4)]
            trib_f = pp.tile([P, P], F32, tag="trib_f")
            trib_t = pp.tile([P, P], BF16, tag="trib_t")
            eye_f = pp.tile([P, P], F32, tag="eye_f")
            eye_t = pp.tile([P, P], BF16, tag="eye_t")
            bq_t = [pp.tile([P, 1], F32, name=f"bq{m}", tag=f"bq{m}") for m in range(4)]
            bk_t = [pp.tile([P, 1], F32, name=f"bk{m}", tag=f"bk{m}") for m in range(4)]
            bv_t = pp.tile([1, 512], F32R, tag="bv")
            ones_t = pp.tile([1, P], F32R, tag="ones")

            nc.gpsimd.dma_start(trib_f[:], trib[:])
            nc.gpsimd.dma_start(eye_f[:], eye[:])
            nc.vector.tensor_copy(trib_t[:], trib_f[:])
            nc.vector.tensor_copy(eye_t[:], eye_f[:])
            nc.vector.memset(ones_t[:].bitcast(F32), 1.0)
            # only the `ones` columns [64:128) of each 192-col pair block need
            # 1.0 -- the V halves are overwritten every iteration.
            for j in range(NT):
                vap = V[j][:]
                part = list(vap.ap)[0]
                nc.vector.memset(
                    AP(vap.tensor, vap.offset + 64, [part, (192, 4), (1, 64)]), 1.0)

            chain = repeat > 1
            tok = None
            for _rep in range(repeat):
                rep_pool_cm = tc.tile_pool(name=f"rp{_rep}", bufs=1)
                rp = rep_pool_cm.__enter__()
                if chain and _rep > 0:
                    # serialize iterations (timing builds): biases depend on
                    # 0 x previous iteration's output.
                    bq_e, bk_e = [], []
                    for m in range(4):
                        t1 = rp.tile([P, 1], F32, name=f"bqe{_rep}_{m}", tag=f"bqe{_rep}_{m}")
                        nc.vector.tensor_add(t1[:], bq_t[m][:], tok[:])
                        bq_e.append(t1)
                        t2 = rp.tile([P, 1], F32, name=f"bke{_rep}_{m}", tag=f"bke{_rep}_{m}")
                        nc.vector.tensor_add(t2[:], bk_t[m][:], tok[:])
                        bk_e.append(t2)
                    bv_e = rp.tile([1, 512], F32R, name=f"bve{_rep}", tag=f"bve{_rep}")
                    nc.vector.tensor_scalar_add(bv_e[:], bv_t[:], tok[0:1, 0:1])
                else:
                    bq_e, bk_e, bv_e = bq_t, bk_t, bv_t

                xpool_cm = tc.tile_pool(name=f"xp{_rep}", bufs=1)
                xp = xpool_cm.__enter__()
                X = [xp.tile([P, T], F32R, name=f"x{k}", tag=f"x{k}") for k in range(CK)]

                # bd: tensors that live through the whole iteration (WV is
                # needed by the deferred V blocks during attention; WPb by
                # the interleaved output projection).
                bd = tc.tile_pool(name="bd", bufs=1)
                bdp = bd.__enter__()
                WV = [bdp.tile([P, 512], F32R, name=f"wv{k}", tag=f"wv{k}") for k in range(CK)]
                WPb = [bdp.tile([P, C], BF16, name=f"wpb{k}", tag=f"wpb{k}")
                       for k in range(4)]

                qtpool = tc.tile_pool(name="qtp", bufs=2)
                qtp = qtpool.__enter__()
                bdps = tc.tile_pool(name="bdps", bufs=2, space="PSUM")
                bdpsp = bdps.__enter__()
                lpool_cm = tc.tile_pool(name="lrec", bufs=2)
                lpool = lpool_cm.__enter__()
                opool_cm = tc.tile_pool(name="ostage", bufs=4)
                opool = opool_cm.__enter__()

                QTs = [None] * 4
                KTs = [None] * 4
                QPs = [None] * 4
                KPs = [None] * 4
                WQp = [None] * 4
                WKp = [None] * 4

                def load_wqwk(p, eng_q, eng_k):
                    # per-pair [128,128] column slices of wq/wk, one tile of
                    # 8 contraction chunks each
                    WQp[p] = qtp.tile([P, CK * P], F32R, name=f"wqp{p}", tag="wqp")
                    WKp[p] = qtp.tile([P, CK * P], F32R, name=f"wkp{p}", tag="wkp")
                    for k in range(CK):
                        eng_q.dma_start(
                            WQp[p][:, k * P:(k + 1) * P],
                            wq[k * P:(k + 1) * P, p * P:(p + 1) * P].bitcast(F32R))
                        eng_k.dma_start(
                            WKp[p][:, k * P:(k + 1) * P],
                            wk[k * P:(k + 1) * P, p * P:(p + 1) * P].bitcast(F32R))

                def pack(dst, src_t):
                    # [128, T] fp8 -> [64, 2, T] double-row packed, one DMA
                    dap = dst[:]
                    part = list(dap.ap)[0]
                    nc.sync.dma_start(
                        AP(dap.tensor, dap.offset, [part, (T, 2), (1, T)]),
                        src_t[:])

                def b_step_micro(p, n):
                    # Q^T/K^T chunk for pair p, token-slice n (x from SBUF),
                    # written as fp8e4 and double-row packed after slab 3.
                    # 8 micro-steps of 2 matmuls (~0.4us) each.
                    cell = {}

                    def mk(which, kk):
                        W, QK, QKP, b_e = (
                            (WQp, QTs, QPs, bq_e) if which == "q"
                            else (WKp, KTs, KPs, bk_e))

                        def f():
                            if which == "q" and kk == 0 and n == 0:
                                QTs[p] = qtp.tile([P, T], FP8, name=f"qtt{p}", tag="qtt")
                                KTs[p] = qtp.tile([P, T], FP8, name=f"ktt{p}", tag="ktt")
                                QPs[p] = qtp.tile([64, 2 * T], FP8, name=f"qpp{p}", tag="qpp")
                                KPs[p] = qtp.tile([64, 2 * T], FP8, name=f"kpp{p}", tag="kpp")
                            if kk == 0:
                                cell[which] = bdpsp.tile([P, 512], F32, name="bps", tag="bps")
                            ps = cell[which]
                            for k in (2 * kk, 2 * kk + 1):
                                nc.tensor.matmul(
                                    ps[:], W[p][:, k * P:(k + 1) * P],
                                    X[k][:, n * 512:(n + 1) * 512],
                                    start=(k == 0), stop=(k == CK - 1),
                                )
                            if kk == 3:
                                nc.vector.tensor_scalar_add(
                                    QK[p][:, n * 512:(n + 1) * 512], ps[:], b_e[p][:])
                                if n == 3:
                                    pack(QKP[p], QK[p])
                        return f

                    return [mk(w, kk) for w in ("q", "k") for kk in range(4)]

                def v_copy_out(i, ps):
                    # interleave psum [tokens, 8*64] into [V_e | ones | V_o]
                    vap = V[i][:]
                    part = list(vap.ap)[0]
                    psap = ps[:]
                    pspart = list(psap.ap)[0]
                    nc.vector.tensor_copy(
                        AP(vap.tensor, vap.offset, [part, (192, 4), (1, 64)]),
                        AP(psap.tensor, psap.offset, [pspart, (128, 4), (1, 64)]))
                    nc.vector.tensor_copy(
                        AP(vap.tensor, vap.offset + 128, [part, (192, 4), (1, 64)]),
                        AP(psap.tensor, psap.offset + 64, [pspart, (128, 4), (1, 64)]))

                def v_block_micro(i, pspool, pstag):
                    # one V token-block as 4 micro-steps
                    cell = {}

                    def mk(kk):
                        def f():
                            if kk == 0:
                                cell["ps"] = pspool.tile([P, 512], F32,
                                                         name="vps", tag=pstag)
                            ps = cell["ps"]
                            for k in (2 * kk, 2 * kk + 1):
                                nc.tensor.matmul(
                                    ps[:], X[k][:, i * P:(i + 1) * P], WV[k][:],
                                    start=(k == 0), stop=False,
                                )
                            if kk == 3:
                                nc.tensor.matmul(ps[:], ones_t[:], bv_e[:],
                                                 start=False, stop=True)
                                v_copy_out(i, ps)
                        return f

                    return [mk(kk) for kk in range(4)]

                def v_stat_ap(j: int, h: int) -> AP:
                    # [V_even|ones|V_odd] per pair: even h -> [V_h | ones]
                    # (O^T rows 0:64, denom rows 64:128); odd h -> [ones | V_h].
                    e = h // 2
                    start = 192 * e + (64 if h % 2 else 0)
                    return V[j][:, start:start + 128]

                # ---- phase C: V blocks 0..7 + pair-0 Q/K projection,
                # interleaved as x slabs arrive ----
                with tc.tile_pool(name="vps", bufs=6, space="PSUM") as vps:
                    # DMA queue assignment (first-needed first):
                    for k in range(6):
                        nc.sync.dma_start(
                            X[k][:, 0:512], xt[k * P:(k + 1) * P, 0:512].bitcast(F32R))
                    for k in range(6):
                        nc.scalar.dma_start(WV[k][:], wv[k * P:(k + 1) * P, :].bitcast(F32R))
                    for k in range(6, CK):
                        nc.gpsimd.dma_start(
                            X[k][:, 0:512], xt[k * P:(k + 1) * P, 0:512].bitcast(F32R))
                    for k in range(6, CK):
                        nc.gpsimd.dma_start(WV[k][:], wv[k * P:(k + 1) * P, :].bitcast(F32R))
                    nc.sync.dma_start(bv_t[:], bv[:].bitcast(F32R))
                    for k in range(4):
                        nc.sync.dma_start(
                            X[k][:, 512:1024], xt[k * P:(k + 1) * P, 512:1024].bitcast(F32R))
                    for k in range(4, CK):
                        nc.scalar.dma_start(
                            X[k][:, 512:1024], xt[k * P:(k + 1) * P, 512:1024].bitcast(F32R))
                    load_wqwk(0, nc.sync, nc.scalar)
                    for m in range(4):
                        nc.sync.dma_start(bq_t[m][:], bq[m * P:(m + 1) * P, :])
                        nc.sync.dma_start(bk_t[m][:], bk[m * P:(m + 1) * P, :])
                    for k in range(CK):
                        nc.scalar.dma_start(
                            X[k][:, 1024:1536], xt[k * P:(k + 1) * P, 1024:1536].bitcast(F32R))
                    for k in range(CK):
                        nc.sync.dma_start(
                            X[k][:, 1536:2048], xt[k * P:(k + 1) * P, 1536:2048].bitcast(F32R))
                    load_wqwk(1, nc.sync, nc.scalar)

                    b0 = [b_step_micro(0, n) for n in range(4)]
                    for i in range(5):
                        for f in v_block_micro(i, vps, "ps"):
                            f()
                    for f in b0[0]:
                        f()
                    for i in (5, 6, 7):
                        for f in v_block_micro(i, vps, "ps"):
                            f()
                        for f in b0[i - 4]:
                            f()

                # attention-only PSUM pools in the banks freed by vps
                spool_cm = tc.tile_pool(name="spsum", bufs=2, space="PSUM")
                spool = spool_cm.__enter__()
                avpool_cm = tc.tile_pool(name="avps", bufs=2, space="PSUM")
                avpool = avpool_cm.__enter__()
                ptpool_cm = tc.tile_pool(name="pt", bufs=2)
                ptpool = ptpool_cm.__enter__()

                def d_group(p, s):
                    # fused pipeline over both heads hh=0,1 of the pair:
                    # units of the two heads alternate, doubling the
                    # independent work between an S matmul and the exp it
                    # feeds (the exp is slower than the matmuls it paces).
                    QP, KP = QPs[p], KPs[p]
                    av = {hh: avpool.tile([P, 512], F32, name=f"av{hh}", tag="av")
                          for hh in range(2)}

                    def s_mm(sp, spc0, hh, j, q0, w, start=True, stop=True):
                        # S^T chunk [128 keys, w queries] via fp8 DoubleRow
                        qb = QP[hh * 32:(hh + 1) * 32, :]
                        kb = KP[hh * 32:(hh + 1) * 32, :]
                        qpart = list(qb.ap)[0]
                        kpart = list(kb.ap)[0]
                        nc.tensor.matmul(
                            sp[:, spc0:spc0 + w],
                            AP(kb.tensor, kb.offset + j * P, [kpart, (T, 2), (1, P)]),
                            AP(qb.tensor, qb.offset + q0, [qpart, (T, 2), (1, w)]),
                            start=start, stop=stop, perf_mode=DOUBLE_ROW,
                        )

                    def tri_mm(sp, spc0, stop):
                        # accumulate the NEG triangle into sp[:, spc0:+128]
                        # via trib^T @ I on the PE (128 cycles)
                        nc.tensor.matmul(
                            sp[:, spc0:spc0 + P], trib_t[:], eye_t[:],
                            start=False, stop=stop,
                        )

                    # units: [full pairs of chunks...] + [diagA, diagB]
                    # full unit jj: chunks 2jj, 2jj+1 (full 512-wide)
                    # diagA: j=4s   @ sp[0:512]  (512w), j=4s+1 @ sp[512:896] (384w)
                    # diagB: j=4s+2 @ sp[0:256]  (256w), j=4s+3 @ sp[256:384] (128w)
                    units = [("full", jj) for jj in range(2 * s)] + [("dA", 0), ("dB", 0)]
                    pts = {0: [], 1: []}

                    def emit_s(hh, u):
                        kind, jj = units[u]
                        sp = spool.tile([P, 1024], F32, tag="sp")
                        if kind == "full":
                            for cc in range(2):
                                j = 2 * jj + cc
                                s_mm(sp, cc * 512, hh, j, s * 512, 512)
                            ew = 1024
                        elif kind == "dA":
                            j0 = 4 * s
                            s_mm(sp, 0, hh, j0, s * 512, 512, stop=False)
                            s_mm(sp, 512, hh, j0 + 1, s * 512 + 128, 384, stop=False)
                            tri_mm(sp, 0, stop=True)      # bank 0
                            tri_mm(sp, 512, stop=True)    # bank 1
                            ew = 896
                        else:  # dB (both chunks + both triangles in bank 0)
                            j2 = 4 * s + 2
                            s_mm(sp, 0, hh, j2, s * 512 + 256, 256, stop=False)
                            s_mm(sp, 256, hh, j2 + 1, s * 512 + 384, 128,
                                 start=False, stop=False)
                            tri_mm(sp, 0, stop=False)
                            tri_mm(sp, 256, stop=True)
                            ew = 384
                        pt = ptpool.tile([P, 1024], BF16, tag="pt")
                        nc.scalar.activation(pt[:, 0:ew], sp[:, 0:ew], AF.Exp, scale=0.125)
                        pts[hh].append(pt)

                    def emit_av(hh, u):
                        h = 2 * p + hh
                        kind, jj = units[u]
                        pt = pts[hh][u]
                        avh = av[hh]
                        if kind == "full":
                            for cc in range(2):
                                j = 2 * jj + cc
                                nc.tensor.matmul(
                                    avh[:], v_stat_ap(j, h),
                                    pt[:, cc * 512:(cc + 1) * 512],
                                    start=(j == 0), stop=False,
                                )
                        elif kind == "dA":
                            j0 = 4 * s
                            nc.tensor.matmul(
                                avh[:], v_stat_ap(j0, h), pt[:, 0:512],
                                start=(j0 == 0), stop=False,
                            )
                            nc.tensor.matmul(
                                avh[:, 128:512], v_stat_ap(j0 + 1, h), pt[:, 512:896],
                                start=False, stop=False,
                            )
                        else:  # dB
                            j2 = 4 * s + 2
                            nc.tensor.matmul(
                                avh[:, 256:512], v_stat_ap(j2, h), pt[:, 0:256],
                                start=False, stop=False,
                            )
                            nc.tensor.matmul(
                                avh[:, 384:512], v_stat_ap(j2 + 1, h), pt[:, 256:384],
                                start=False, stop=True,
                            )

                    def finish(hh):
                        # evict av to SBUF at once (frees the PSUM slot for
                        # the next group's AV), then normalize off the
                        # critical path: reciprocal of the denominator rows,
                        # DMA-shift to O^T's partitions, multiply into YT.
                        prow = hh * 64
                        lrow = 64 - prow
                        avs = lpool.tile([P, 512], F32, tag="avs")
                        nc.vector.tensor_copy(avs[:], av[hh][:])
                        rec = lpool.tile([P, 512], F32, tag="rec")
                        nc.vector.reciprocal(
                            rec[lrow:lrow + 64, :], avs[lrow:lrow + 64, :])
                        rec2 = lpool.tile([P, 512], F32, tag="rec2")
                        nc.gpsimd.dma_start(
                            rec2[prow:prow + 64, :], rec[lrow:lrow + 64, :])
                        nc.vector.tensor_mul(
                            YT[p][prow:prow + 64, s * 512:(s + 1) * 512],
                            avs[prow:prow + 64, :],
                            rec2[prow:prow + 64, :],
                        )

                    nu = len(units)
                    seq = [(hh, u) for u in range(nu) for hh in range(2)]
                    emit_s(*seq[0])
                    yield
                    emit_s(*seq[1])
                    yield
                    for i in range(2, len(seq)):
                        emit_s(*seq[i])
                        emit_av(*seq[i - 2])
                        yield
                    emit_av(*seq[-2])
                    emit_av(*seq[-1])
                    finish(0)
                    finish(1)

                # ---- phase E: output projection, interleaved into pair-3's
                # attention. Uses the b_step PSUM tiles.
                last_ost = [None]

                def load_wp():
                    for k in range(4):
                        for h2 in range(2):
                            wpf = lpool.tile([P, 512], F32, name=f"wpf{k}_{h2}", tag="wpf")
                            nc.sync.dma_start(
                                wpf[:], wp[k * P:(k + 1) * P, h2 * 512:(h2 + 1) * 512])
                            nc.gpsimd.tensor_copy(
                                WPb[k][:, h2 * 512:(h2 + 1) * 512], wpf[:])

                def e_step(m, nn, pspool, pstag, dma_eng=None, copy_act=False):
                    def f():
                        ps = pspool.tile([P, 512], F32, name="eps", tag=pstag)
                        for kf in range(4):
                            nc.tensor.matmul(
                                ps[:],
                                YT[kf][:, m * P:(m + 1) * P],
                                WPb[kf][:, nn * 512:(nn + 1) * 512],
                                start=(kf == 0), stop=(kf == 3),
                            )
                        ost = opool.tile([P, 512], F32, tag="ost")
                        if copy_act:
                            # tail only: ACT is idle after the last exp
                            nc.scalar.activation(ost[:], ps[:], AF.Copy)
                        else:
                            nc.vector.tensor_copy(ost[:], ps[:])
                        (dma_eng or nc.sync).dma_start(
                            out[m * P:(m + 1) * P, nn * 512:(nn + 1) * 512], ost[:])
                        last_ost[0] = ost
                    return f

                # master emission: per pair, attention units with all other
                # PE work interleaved as micro-step padding.
                from collections import deque
                pads = deque()

                def pump():
                    n = 2 if len(pads) > 24 else 1
                    for _ in range(n):
                        if pads:
                            pads.popleft()()

                def run_group(p, s):
                    for _ in d_group(p, s):
                        pump()

                load_wqwk(2, nc.gpsimd, nc.gpsimd)
                for i in range(8, NT):
                    pads.extend(v_block_micro(i, bdpsp, "bps"))
                for n in range(4):
                    pads.extend(b_step_micro(1, n))
                for s in (0, 1, 2, 3):
                    run_group(0, s)
                while pads:
                    pads.popleft()()

                load_wqwk(3, nc.gpsimd, nc.gpsimd)
                for n in range(4):
                    pads.extend(b_step_micro(2, n))
                for s in (3, 2, 1, 0):
                    run_group(1, s)
                while pads:
                    pads.popleft()()

                for n in range(4):
                    pads.extend(b_step_micro(3, n))
                for s in (3, 2, 1, 0):
                    run_group(2, s)
                while pads:
                    pads.popleft()()

                load_wp()
                for s in (3, 2, 1, 0):
                    run_group(3, s)
                    for m in range(4 * s + 3, 4 * s - 1, -1):
                        if s > 0:
                            pads.append(e_step(m, 0, bdpsp, "bps"))
                            pads.append(e_step(m, 1, bdpsp, "bps"))
                while pads:
                    pads.popleft()()

                # tail: close the attention PSUM pools, run the last output
                # blocks with a 4-deep PSUM pipeline in the freed banks.
                ptpool_cm.__exit__(None, None, None)
                avpool_cm.__exit__(None, None, None)
                spool_cm.__exit__(None, None, None)
                with tc.tile_pool(name="projps", bufs=4, space="PSUM") as prpool:
                    for m in (3, 2, 1, 0):
                        e_step(m, 0, prpool, "pp", nc.sync, copy_act=True)()
                        e_step(m, 1, prpool, "pp", nc.gpsimd)()

                    if chain:
                        tok = pp.tile([P, 1], F32, name=f"tok{_rep}", tag=f"tok{_rep}")
                        nc.vector.tensor_scalar_mul(tok[:], last_ost[0][:, 0:1], 0.0)
                    if timing and _rep == repeat - 1:
                        nc.sync.dma_start(probe[:], last_ost[0][:, 0:4])

                opool_cm.__exit__(None, None, None)
                lpool_cm.__exit__(None, None, None)
                bdps.__exit__(None, None, None)
                qtpool.__exit__(None, None, None)
                bd.__exit__(None, None, None)
                xpool_cm.__exit__(None, None, None)
                rep_pool_cm.__exit__(None, None, None)

    nc.compile()
    return nc


_TRI = np.where(
    np.arange(P)[:, None] > np.arange(P)[None, :],
    np.float32(NEG), np.float32(0.0),
).astype(np.float32)
_EYE = np.eye(P, dtype=np.float32)


def shard_inputs(x, w_attn, b_attn, w_proj, b_proj):
    """Build the per-core input maps."""
    x = np.asarray(x, dtype=np.float32)
    w_attn = np.asarray(w_attn, dtype=np.float32)
    b_attn = np.asarray(b_attn, dtype=np.float32)
    w_proj = np.asarray(w_proj, dtype=np.float32)
    in_maps = []
    for c in range(N_CORES):
        b, g = divmod(c, 2)
        sl = slice(g * 512, (g + 1) * 512)
        in_maps.append({
            "xt": np.ascontiguousarray(x[b].T),
            "wq": np.ascontiguousarray(w_attn[:, g * 512:(g + 1) * 512]),
            "wk": np.ascontiguousarray(w_attn[:, 1024 + g * 512:1024 + (g + 1) * 512]),
            "wv": np.ascontiguousarray(w_attn[:, 2048 + g * 512:2048 + (g + 1) * 512]),
            "wp": np.ascontiguousarray(w_proj[g * 512:(g + 1) * 512, :]),
            "bq": np.ascontiguousarray(b_attn[sl].reshape(512, 1)),
            "bk": np.ascontiguousarray(b_attn[1024 + g * 512:1024 + (g + 1) * 512].reshape(512, 1)),
            "bv": np.ascontiguousarray(b_attn[2048 + g * 512:2048 + (g + 1) * 512].reshape(1, 512)),
            "trib": np.ascontiguousarray(_TRI.T),
            "eye": _EYE,
        })
    return in_maps


def gather_output(results, b_proj):
    b_proj = np.asarray(b_proj, dtype=np.float32)
    out = np.empty((4, T, C), dtype=np.float32)
    for b in range(4):
        out[b] = results[2 * b]["out"] + results[2 * b + 1]["out"] + b_proj
    return out


_NC_CACHE = None


def get_nc():
    global _NC_CACHE
    if _NC_CACHE is None:
        _NC_CACHE = build_nc()
    return _NC_CACHE


def kernel(x, w_attn, b_attn, w_proj, b_proj):
    from concourse.bass_utils import run_bass_kernel_spmd

    in_maps = shard_inputs(x, w_attn, b_attn, w_proj, b_proj)
    nc = get_nc()
    res = run_bass_kernel_spmd(nc, in_maps, list(range(N_CORES)))
    return gather_output(res.results, b_proj)
